# revision 1
# baseline (speedup 1.0000x reference)
"""Causal self-attention (B=4, T=2048, C=1024, H=16) on 8 trn2 NeuronCores.

Sharding: head-parallel tensor parallelism. Each core owns 2 of the 16 heads:
 - QKV projection computed for its 384 rows of Wqkv (2 heads x 64 x {q,k,v})
 - attention for its 2 heads (causal, block-skipped)
 - partial out-projection against its 128 columns of Wout
The 8 partial [C, B*T] outputs are summed on the host (the "all-reduce").

Device layouts (chosen so every matmul contraction dim lands on SBUF
partitions with zero on-device transposes except a cheap 128x128 PE
transpose of V):
  xT    [C, B*T]   x transposed on host
  qT/kT/vT [128=2h*64, T]  per batch, produced by the QKV matmuls
  S^T   [tk, tq]   scores transposed => softmax row-sum over partitions is a
                   ones-matmul; PV needs exactly this layout
  outT  [C, B*T]   partial output, transposed back on host

Softmax row-sums ride inside the PV matmuls: the stationary operand is
[v_h | pad_bcast], so PSUM rows 0:64 accumulate Y_h and rows 64:128 the
pad-masked row-sum (replicated). A constant "swap" matmul mirrors the two
row-sum halves onto the opposite partition halves so the normalize multiply
is partition-aligned.

All matmuls run as float32r (fp32 read, fp22 multiply) which is full
TensorE rate (1 cycle/row) at free-dim >= 256, ~4x faster than true fp32.
fp32r matmuls cannot target PSUM partition offsets (col tile_position != 0
fails walrus codegen), which is why no column packing is used.
"""

import numpy as np
from contextlib import ExitStack

import concourse.bass as bass
import concourse.bacc as bacc
import concourse.mybir as mybir
import concourse.tile as tile
from concourse import bass_utils
from concourse.masks import make_identity

B, T, C = 4, 2048, 1024
H, D = 16, 64
NCORES = 8
HPC = H // NCORES            # heads per core = 2
CPC = HPC * D                # y-channels per core = 128
BT = B * T                   # 8192
F = 3 * CPC                  # qkv rows per core = 384
TQB = 512                    # tq block (matmul free dim)
NJ = T // TQB                # 4 tq blocks per batch
NKT = T // 128               # 16 tk tiles per batch
NCT = C // 128               # 8 contraction tiles for projections
FP32 = mybir.dt.float32
FP32R = mybir.dt.float32r
AF = mybir.ActivationFunctionType
SCALE = 1.0 / np.sqrt(D)

_cached = {}

# build-time config knobs (A/B testing)
CFG = {
    "batched_dma": False,    # per-c-tile DMAs beat one 3D DMA on HW
    "mask_engine": "vector", # DVE mask beats gpsimd (pool 2-input is slow)
    "spsum_bufs": 2,
    "accps_bufs": 1,
    "exp_mode": "combined",     # "split": per-head [128,512] exp; "combined": one [128,1024] exp

    "qkps_bufs": 2,
    "ppool_bufs": 8,
    "bias_engine": "scalar",  # "scalar": ACT Identity+bias; "vector": DVE add
}


def _emit(tc, nc, xT, wqkvT, bqkv, woutT, padT, outT, reps=1):
    ctx = ExitStack()
    with ctx:
        const = ctx.enter_context(tc.tile_pool(name="const", bufs=1))
        xpool = ctx.enter_context(tc.tile_pool(name="xpool", bufs=2))
        qkvpool = ctx.enter_context(tc.tile_pool(name="qkvpool", bufs=2))
        ppool = ctx.enter_context(tc.tile_pool(name="ppool", bufs=CFG["ppool_bufs"]))
        ypool = ctx.enter_context(tc.tile_pool(name="ypool", bufs=2))
        opool = ctx.enter_context(tc.tile_pool(name="opool", bufs=3))
        spsum = ctx.enter_context(tc.tile_pool(name="spsum", bufs=CFG["spsum_bufs"], space="PSUM"))
        accps = ctx.enter_context(tc.tile_pool(name="accps", bufs=CFG["accps_bufs"], space="PSUM"))
        qkps = ctx.enter_context(tc.tile_pool(name="qkps", bufs=CFG["qkps_bufs"], space="PSUM"))

        # ---- constants ----
        identity = const.tile([128, 128], FP32)
        make_identity(nc, identity)
        # swap matrix: mirrors partition halves (and scales by 1/64 to undo
        # the 64-fold replication summed by the swap matmul). Built in fp32,
        # then copied through DVE so the fp32r operand counts as rounded.
        swap_f32 = const.tile([128, 128], FP32)
        nc.vector.memset(swap_f32, 0.0)
        nc.vector.memset(swap_f32[0:64, 64:128], 1.0 / 64.0)
        nc.vector.memset(swap_f32[64:128, 0:64], 1.0 / 64.0)
        swapm = const.tile([128, 128], FP32R)
        nc.vector.tensor_copy(swapm, swap_f32)
        # 4 diagonal-block causal masks, each replicated for the 2 heads:
        # mask2[m][p, h*512 + q] = 1.0 if p <= q - 128*m else 0.0
        mask2 = []
        for m in range(4):
            mk = const.tile([128, 2 * TQB], mybir.dt.bfloat16, name=f"mask2_{m}")
            nc.gpsimd.memset(mk, 1.0)
            for h in range(2):
                nc.gpsimd.affine_select(
                    out=mk[:, h * TQB:(h + 1) * TQB],
                    in_=mk[:, h * TQB:(h + 1) * TQB],
                    compare_op=mybir.AluOpType.is_ge,
                    fill=0.0,
                    base=-128 * m,
                    pattern=[[1, TQB]],
                    channel_multiplier=-1,
                )
            mask2.append(mk)

        # weights
        w_sb = const.tile([128, NCT, F], FP32R)     # wqkvT tiles: [c-tile][f]
        for ct in range(NCT):
            nc.sync.dma_start(w_sb[:, ct, :], wqkvT[ct * 128:(ct + 1) * 128, :])
        b_sb = const.tile([128, 3], FP32)
        for ft in range(3):
            nc.gpsimd.dma_start(b_sb[:, ft:ft + 1],
                                bqkv[ft * 128:(ft + 1) * 128].unsqueeze(1))
        wo_sb = const.tile([128, C], FP32R)         # woutT [cy, o]
        nc.sync.dma_start(wo_sb, woutT)
        bb_sb = None
        if CFG["bias_engine"] == "vector":
            # bias broadcast along free dim, built once on ACT (zero input +
            # per-partition bias), consumed by DVE adds in steady state
            zb = const.tile([128, TQB], FP32)
            nc.vector.memset(zb, 0.0)
            bb_sb = const.tile([128, 3, TQB], FP32)
            for ft in range(3):
                nc.scalar.activation(bb_sb[:, ft, :], zb, AF.Identity,
                                     bias=b_sb[:, ft:ft + 1])

        for rep in range(reps):
            for b in range(B):
                # ---- QKV projection for this batch: qT/kT/vT [128, T] ----
                qkv_sb = qkvpool.tile([128, 3, T], FP32R, name=f"{rep}_qkv_{b}", tag="qkv")
                for jj in range(NJ):
                    tb = b * NJ + jj
                    x_sb = xpool.tile([128, NCT, TQB], FP32R, name=f"{rep}_x_{tb}", tag="x")
                    if CFG["batched_dma"]:
                        nc.sync.dma_start(
                            x_sb,
                            xT[:, tb * TQB:(tb + 1) * TQB].rearrange(
                                "(ct p) q -> p ct q", p=128))
                    else:
                        for ct in range(NCT):
                            nc.sync.dma_start(
                                x_sb[:, ct, :],
                                xT[ct * 128:(ct + 1) * 128,
                                   tb * TQB:(tb + 1) * TQB])
                    for ft in range(3):
                        ps = qkps.tile([128, TQB], FP32, name=f"{rep}_qkvps_{tb}_{ft}",
                                       tag="qk")
                        for ct in range(NCT):
                            nc.tensor.matmul(
                                ps,
                                lhsT=w_sb[:, ct, ft * 128:(ft + 1) * 128],
                                rhs=x_sb[:, ct, :],
                                start=(ct == 0), stop=(ct == NCT - 1))
                        # bias-add + copy to SBUF (Identity is resident in
                        # every ACT table set, incl. exp's)
                        if CFG["bias_engine"] == "vector":
                            nc.vector.tensor_add(
                                qkv_sb[:, ft, jj * TQB:(jj + 1) * TQB], ps,
                                bb_sb[:, ft, :])
                        else:
                            nc.scalar.activation(
                                qkv_sb[:, ft, jj * TQB:(jj + 1) * TQB], ps,
                                AF.Identity, bias=b_sb[:, ft:ft + 1])
                q_sb = qkv_sb[:, 0, :]
                k_sb = qkv_sb[:, 1, :]
                vT_sb = qkv_sb[:, 2, :]

                # pad value replicated along the free dim (host pre-broadcast):
                # pbc[p, i, f] = pad[b, i*128 + p]
                pbc_sb = qkvpool.tile([128, NKT, 128], FP32R, name=f"{rep}_pbc_{b}",
                                      tag="pbc")
                nc.sync.dma_start(
                    pbc_sb, padT[b].rearrange("(i p) f -> p i f", p=128))

                # ---- transpose V to [tk, d]; build augmented PV stationaries
                #      vA = [v_h0 * pad | pad], vB = [pad | v_h1 * pad]
                # The pad halves are DMAed straight from DRAM (no DVE copies).
                vA_sb = qkvpool.tile([128, NKT, 128], FP32R, name=f"{rep}_vA_{b}",
                                     tag="vA")
                vB_sb = qkvpool.tile([128, NKT, 128], FP32R, name=f"{rep}_vB_{b}",
                                     tag="vB")
                nc.sync.dma_start(
                    vA_sb[:, :, 64:128],
                    padT[b, :, 0:64].rearrange("(i p) f -> p i f", p=128))
                nc.sync.dma_start(
                    vB_sb[:, :, 0:64],
                    padT[b, :, 0:64].rearrange("(i p) f -> p i f", p=128))
                for i in range(NKT):
                    pvt = qkps.tile([128, 128], FP32, name=f"{rep}_vt_{b}_{i}", tag="qk")
                    nc.tensor.transpose(pvt,
                                        vT_sb[:, i * 128:(i + 1) * 128].bitcast(
                                            FP32),
                                        identity)
                    nc.vector.tensor_mul(vA_sb[:, i, 0:64], pvt[:, 0:64],
                                         pbc_sb[:, i, 0:64])
                    nc.vector.tensor_mul(vB_sb[:, i, 64:128], pvt[:, 64:128],
                                         pbc_sb[:, i, 64:128])

                # ---- attention per tq block ----
                for j in range(NJ):
                    ntk = 4 * (j + 1)
                    pyA = accps.tile([128, TQB], FP32, name=f"{rep}_pyA_{b}_{j}",
                                     tag="pyA")
                    pyB = accps.tile([128, TQB], FP32, name=f"{rep}_pyB_{b}_{j}",
                                     tag="pyB")
                    for i in range(ntk):
                        p_sb = ppool.tile([128, 2 * TQB], FP32R,
                                          name=f"{rep}_p_{b}_{j}_{i}", tag="p")
                        if CFG["exp_mode"] == "combined":
                            ps2 = spsum.tile([128, 2 * TQB], FP32,
                                             name=f"{rep}_s_{b}_{j}_{i}",
                                             tag="s")
                            for h in range(2):
                                nc.tensor.matmul(
                                    ps2[:, h * TQB:(h + 1) * TQB],
                                    lhsT=k_sb[h * 64:(h + 1) * 64,
                                              i * 128:(i + 1) * 128],
                                    rhs=q_sb[h * 64:(h + 1) * 64,
                                             j * TQB:(j + 1) * TQB],
                                    start=True, stop=True,
                                    tile_position=(h * 64, 0))
                            nc.scalar.activation(p_sb, ps2, AF.Exp,
                                                 scale=float(SCALE))
                        else:
                            for h in range(2):
                                ps = spsum.tile([128, TQB], FP32,
                                                name=f"{rep}_s_{b}_{j}_{i}_{h}",
                                                tag="s")
                                nc.tensor.matmul(
                                    ps,
                                    lhsT=k_sb[h * 64:(h + 1) * 64,
                                              i * 128:(i + 1) * 128],
                                    rhs=q_sb[h * 64:(h + 1) * 64,
                                             j * TQB:(j + 1) * TQB],
                                    start=True, stop=True,
                                    tile_position=(h * 64, 0))
                                nc.scalar.activation(
                                    p_sb[:, h * TQB:(h + 1) * TQB], ps,
                                    AF.Exp, scale=float(SCALE))
                        if i >= 4 * j:
                            # gpsimd: DVE is the busiest engine, Pool is idle,
                            # and all three operands are SBUF (Pool can't touch
                            # PSUM)
                            if CFG["mask_engine"] == "pool":
                                nc.gpsimd.tensor_mul(p_sb, p_sb,
                                                     mask2[i - 4 * j])
                            else:
                                nc.vector.tensor_mul(p_sb, p_sb,
                                                     mask2[i - 4 * j])
                        first, last = (i == 0), (i == ntk - 1)
                        # rows 0:64 <- Y_h0, rows 64:128 <- rowsum_h0 (x64)
                        nc.tensor.matmul(pyA, lhsT=vA_sb[:, i, :],
                                         rhs=p_sb[:, 0:TQB],
                                         start=first, stop=last)
                        # rows 0:64 <- rowsum_h1 (x64), rows 64:128 <- Y_h1
                        nc.tensor.matmul(pyB, lhsT=vB_sb[:, i, :],
                                         rhs=p_sb[:, TQB:2 * TQB],
                                         start=first, stop=last)

                    # assemble [rowsum_h1 | rowsum_h0] and mirror the halves so
                    # each Y row sees its own head's row-sum
                    rs_sb = ypool.tile([128, TQB], FP32R, name=f"{rep}_rs_{b}_{j}",
                                       tag="rs")
                    nc.vector.tensor_copy(rs_sb[0:64, :], pyB[0:64, :])
                    nc.vector.tensor_copy(rs_sb[64:128, :], pyA[64:128, :])
                    prs = qkps.tile([128, TQB], FP32, name=f"{rep}_prs_{b}_{j}",
                                    tag="qk")
                    nc.tensor.matmul(prs, lhsT=swapm, rhs=rs_sb, start=True,
                                     stop=True)
                    recip = ypool.tile([128, TQB], FP32, name=f"{rep}_rc_{b}_{j}",
                                       tag="rc")
                    nc.vector.reciprocal(recip, prs)
                    y_sb = ypool.tile([128, TQB], FP32R, name=f"{rep}_y_{b}_{j}",
                                      tag="y")
                    nc.vector.tensor_mul(y_sb[0:64, :], pyA[0:64, :],
                                         recip[0:64, :])
                    nc.vector.tensor_mul(y_sb[64:128, :], pyB[64:128, :],
                                         recip[64:128, :])

                    # ---- out projection for this tq block ----
                    if CFG["batched_dma"]:
                        for og in range(2):
                            o_sb = opool.tile([128, NCT // 2, TQB], FP32,
                                              name=f"{rep}_o_{b}_{j}_{og}",
                                              tag="o")
                            for oi in range(NCT // 2):
                                ot = og * (NCT // 2) + oi
                                po = qkps.tile([128, TQB], FP32,
                                               name=f"{rep}_po_{b}_{j}_{ot}",
                                               tag="qk")
                                nc.tensor.matmul(
                                    po,
                                    lhsT=wo_sb[:, ot * 128:(ot + 1) * 128],
                                    rhs=y_sb, start=True, stop=True)
                                nc.vector.tensor_copy(o_sb[:, oi, :], po)
                            nc.sync.dma_start(
                                outT[og * 512:(og + 1) * 512,
                                     b * T + j * TQB:b * T + (j + 1) * TQB]
                                .rearrange("(ot p) q -> p ot q", p=128), o_sb)
                    else:
                        for ot in range(NCT):
                            po = qkps.tile([128, TQB], FP32,
                                           name=f"{rep}_po_{b}_{j}_{ot}",
                                           tag="qk")
                            nc.tensor.matmul(
                                po, lhsT=wo_sb[:, ot * 128:(ot + 1) * 128],
                                rhs=y_sb, start=True, stop=True)
                            o_sb = opool.tile([128, TQB], FP32,
                                              name=f"{rep}_o_{b}_{j}_{ot}",
                                              tag="o")
                            nc.vector.tensor_copy(o_sb, po)
                            nc.sync.dma_start(
                                outT[ot * 128:(ot + 1) * 128,
                                     b * T + j * TQB:b * T + (j + 1) * TQB],
                                o_sb)


def build(reps=1):
    nc = bacc.Bacc()
    xT = nc.dram_tensor("xT", [C, BT], FP32R, kind="ExternalInput")
    wqkvT = nc.dram_tensor("wqkvT", [C, F], FP32R, kind="ExternalInput")
    bqkv = nc.dram_tensor("bqkv", [F], FP32, kind="ExternalInput")
    woutT = nc.dram_tensor("woutT", [CPC, C], FP32R, kind="ExternalInput")
    padT = nc.dram_tensor("padT", [B, T, 128], FP32R, kind="ExternalInput")
    outT = nc.dram_tensor("outT", [C, BT], FP32, kind="ExternalOutput")
    with tile.TileContext(nc) as tc:
        _emit(tc, nc, xT.ap(), wqkvT.ap(), bqkv.ap(), woutT.ap(), padT.ap(),
              outT.ap(), reps=reps)
    nc.compile()
    return nc


def make_in_maps(x, attention_mask, Wqkv, bqkv, Wout):
    xT = np.ascontiguousarray(
        x.reshape(BT, C).T.astype(np.float32, copy=False))
    padT = np.ascontiguousarray(np.broadcast_to(
        attention_mask.astype(np.float32)[:, :, None], (B, T, 128)))
    in_maps = []
    for c in range(NCORES):
        rows = np.r_[c * CPC:(c + 1) * CPC,
                     C + c * CPC:C + (c + 1) * CPC,
                     2 * C + c * CPC:2 * C + (c + 1) * CPC]
        wqkvT_c = np.ascontiguousarray(Wqkv[rows, :].T.astype(np.float32,
                                                              copy=False))
        b_c = np.ascontiguousarray(bqkv[rows].astype(np.float32, copy=False))
        woutT_c = np.ascontiguousarray(
            Wout[:, c * CPC:(c + 1) * CPC].T.astype(np.float32, copy=False))
        in_maps.append({"xT": xT, "wqkvT": wqkvT_c, "bqkv": b_c,
                        "woutT": woutT_c, "padT": padT})
    return in_maps


def kernel(x, attention_mask, Wqkv, bqkv, Wout, _trace=False):
    x = np.asarray(x)
    attention_mask = np.asarray(attention_mask)
    Wqkv = np.asarray(Wqkv)
    bqkv = np.asarray(bqkv)
    Wout = np.asarray(Wout)
    if "nc" not in _cached:
        _cached["nc"] = build()
    nc = _cached["nc"]
    in_maps = make_in_maps(x, attention_mask, Wqkv, bqkv, Wout)
    res = bass_utils.run_bass_kernel_spmd(
        nc, in_maps, core_ids=list(range(NCORES)), trace=_trace)
    acc = res.results[0]["outT"].astype(np.float32)
    for r in res.results[1:]:
        acc += r["outT"]
    out = np.ascontiguousarray(acc.T).reshape(B, T, C).astype(np.float32)
    if _trace:
        _cached["last_result"] = res
    return out

